# revision 37
# baseline (speedup 1.0000x reference)
"""Multi-head attention (B=2, S=2048, D=1024, H=16) on 8 Trainium2 cores.

Sharding: core = 4*b + g  (b = batch 0..1, g = head-group 0..3, 4 heads each).

Single fused pipeline per core:
  - inputs host-relaid to s-major 1MB blocks [sb, p, c, s]; all input DMA
    drains through one HWDGE FIFO at ~400GB/s with ~7us startup, so every
    DMA-gated compute unit is stamped with tile_wait_until at its realistic
    arrival time — this keeps the Tile scheduler's static per-engine order
    feasible at runtime (its own DMA model has no shared-bandwidth cap);
  - attention runs per (q-chunk, head-pair) k-sweeps, software-pipelined at
    creation: tick t emits ST+exp(t) then PV(t-1), so the next sweep's first
    scores overlap the previous sweep's last PV drain (no ACT bubble);
  - ST = scores^T via 64-contraction matmuls, the two heads of a pair issued
    adjacently on PE row-groups (0,0)/(64,0) -> concurrent execution;
  - exp on ACT is the steady-state bottleneck (128 instrs x ~1.11us);
  - PV accumulates U_h [65, q] in PSUM (row 64 = softmax denominator via a
    ones column in V_aug); U evacuated to SBUF right after each sweep so the
    2 U banks recycle; reciprocals batched per q-chunk;
  - projection / out-projection units interleave as PE filler (HAM stays
    warm); V bias folded out algebraically (host adds W_o@b_v + b_o).

All matmuls in bfloat16 (1 cycle/row, FWL weight loads); fp32 PSUM accum.
"""

import os
from contextlib import ExitStack

import ml_dtypes
import numpy as np

import concourse.bass as bass
import concourse.tile as tile
from concourse import bacc, mybir

B, S, D = 2, 2048, 1024
H, DH = 16, 64
NCORES = 8
NG = 4                  # head-group shards
DG = D // NG            # 256 dims per head-group (4 heads)
P = 128
QC = 512                # q-chunk width
NQC = S // QC           # 4
NKT = S // P            # 16 k-tiles of 128
NSB = S // QC           # 4 s-blocks per input
CD = D // P             # 8 contraction chunks
F32 = mybir.dt.float32
BF16 = mybir.dt.bfloat16
AF = mybir.ActivationFunctionType
SCALE = 1.0 / float(np.sqrt(D))

# DMA arrival estimates (us): single FIFO queue, ~0.4 MB/us, ~9us startup.
_T0, _RATE = 9.0, 0.4


def _body(ctx: ExitStack, tc: "tile.TileContext", io: dict):
    nc = tc.nc
    ctx.enter_context(nc.allow_low_precision(reason="bf16 matmul pipeline"))
    sb = ctx.enter_context(tc.tile_pool(name="sb", bufs=1))
    ps = ctx.enter_context(tc.tile_pool(name="ps", bufs=1, space="PSUM"))

    # ---- input DMAs in FIFO priority order; track cumulative-MB ETAs ------
    eta = {}
    cum = [0.0]

    def ldma(nm, shape, dt, src, mb, nsplit=1):
        t = sb.tile(shape, dt, tag=nm, bufs=1, name=nm)
        if nsplit == 1:
            nc.sync.dma_start(t[:], src)
        else:
            step = shape[1] // nsplit
            for i in range(nsplit):
                sl = slice(i * step, (i + 1) * step)
                nc.sync.dma_start(t[:, sl], src[:, sl])
        cum[0] += mb
        eta[nm] = _T0 + cum[0] / _RATE
        return t

    ones4 = ldma("ones4", [P, 4], BF16, io["ones4"], 0.01)
    wk = ldma("wk", [P, CD, DG], BF16, io["wk"], 0.5)
    xk_t = [ldma("xk0", [P, CD, QC], BF16, io["xk"][0], 1.0, nsplit=2)]
    wq = ldma("wq", [P, CD, DG], BF16, io["wq"], 0.5)
    xq_t = [ldma("xq0", [P, CD, QC], BF16, io["xq"][0], 1.0, nsplit=2)]
    bk = ldma("bk", [P, 2], F32, io["bk"], 0.01)
    bq = ldma("bq", [P, 2], F32, io["bq"], 0.01)
    # interleave xv with xk so V projections flow with sweep 0's k-sweep
    # instead of bunching into a PE backlog after all of xk.
    wv = ldma("wv", [P, CD, DG], BF16, io["wv"], 0.5)
    xv_t = [ldma("xv0", [P, CD, QC], BF16, io["xv"][0], 1.0)]
    for b_ in range(1, NSB):
        xk_t.append(ldma(f"xk{b_}", [P, CD, QC], BF16, io["xk"][b_], 1.0))
        xv_t.append(ldma(f"xv{b_}", [P, CD, QC], BF16, io["xv"][b_], 1.0))
    for b_ in range(1, NSB):
        xq_t.append(ldma(f"xq{b_}", [P, CD, QC], BF16, io["xq"][b_], 1.0))
    woT = [
        ldma(f"wo{pr}", [P, D], BF16, io["wo"][pr * P : (pr + 1) * P, :], 0.25)
        for pr in range(2)
    ]

    # ACT table preload: tiny exp on the first-arriving tile triggers the
    # one-time ~2.7us ACT_TABLE_LOAD while DMAs are still streaming.
    scr = sb.tile([P, 4], BF16, tag="scr", bufs=1, name="scr")
    nc.scalar.activation(scr[:], ones4[:], AF.Exp, scale=SCALE)

    QT, KT, Vt, UN, YSB = {}, {}, {}, {}, {}

    # ---- PE filler units, stamped with realistic DMA-arrival times --------
    def qk_unit(which, d, sc):
        w, xs, bias, outmap = (
            (wq, xq_t, bq, QT) if which == "q" else (wk, xk_t, bk, KT)
        )
        pg = ps.tile([P, QC], F32, tag="fil", bufs=2, name=f"pg_{which}{d}{sc}")
        for c in range(CD):
            nc.tensor.matmul(
                pg[:],
                (w[:, c, d * P : (d + 1) * P]),
                (xs[sc][:, c, :]),
                start=(c == 0),
                stop=(c == CD - 1),
            )
        t = sb.tile([P, QC], BF16, tag=f"{which}t", bufs=8, name=f"{which}T{d}{sc}")
        nc.vector.tensor_scalar_add(t[:], pg[:], bias[:, d : d + 1])
        outmap[d, sc] = t

    def v_unit(kt):
        blk, off = divmod(kt, 4)
        pg = ps.tile([P, DG], F32, tag="fil", bufs=2, name=f"pg_v{kt}")
        for c in range(CD):
            nc.tensor.matmul(
                pg[:],
                (xv_t[blk][:, c, off * P : (off + 1) * P]),
                (wv[:, c, :]),
                start=(c == 0),
                stop=(c == CD - 1),
            )
        vt = sb.tile([P, 4, DH + 1], BF16, tag="v", bufs=16, name=f"V{kt}")
        nc.vector.tensor_copy(
            vt[:, :, 0:DH], pg[:].rearrange("p (g d) -> p g d", g=4)
        )
        nc.vector.tensor_copy(vt[:, :, DH : DH + 1], ones4[:, :, None])
        Vt[kt] = vt

    def outproj_unit(qt, ec, use_act=False):
        qcp, qi = divmod(qt, 4)
        if ec == 0:
            YSB[qt] = sb.tile([P, D], BF16, tag="y", bufs=3, name=f"Y{qt}")
        ysb = YSB[qt]
        yp = ps.tile([P, QC], F32, tag="fil", bufs=2, name=f"yp{qt}_{ec}")
        for pr in range(2):
            nc.tensor.matmul(
                yp[:],
                (UN[qcp, pr][:, qi * P : (qi + 1) * P]),
                (woT[pr][:, ec * QC : (ec + 1) * QC]),
                start=(pr == 0),
                stop=(pr == 1),
            )
        if use_act:
            # tail: ACT is idle after the last exp; offload the PSUM->SBUF
            # evacuation there so DVE and ACT drain the tail in parallel.
            nc.scalar.activation(ysb[:, ec * QC : (ec + 1) * QC], yp[:], AF.Copy)
        else:
            nc.vector.tensor_copy(ysb[:, ec * QC : (ec + 1) * QC], yp[:])
        if ec == 1:
            nc.sync.dma_start(io["y"][qt * P : (qt + 1) * P, :], ysb[:])

    def stamped(us, fn):
        with tc.tile_wait_until(us / 1000.0):
            fn()

    # All units created upfront; stamps = max(DMA arrival, consumer-need
    # minus slack) so the scheduler neither front-loads them into the
    # exp-chain ramp nor bunches them ahead of late DMA arrivals.
    # Hand-scheduled stamp table (scheduler-sim coordinates, us). Invariants:
    # critical d0 K/Q units at their block's DMA arrival; d1 fillers in the
    # following DMA-stall window (never ahead of the exp-chain ramp); V at
    # xv arrival so PVs track the sweep; later-qc Q spread into sweep-1 slack.
    # CRITICAL: all units share the 2-slot "fil" PSUM tag, and slot rotation
    # follows CREATION order — so units must be created in stamp order or the
    # slot chain serializes them in the wrong sequence.
    UNIT_STAMPS = [
        (("k", 0, 0), 12.8),
        (("q", 0, 0), 16.5),
        (("k", 1, 0), 19.4),
        (("v", 0), 20.5),
        (("v", 1), 20.6),
        (("v", 2), 20.7),
        (("v", 3), 20.8),
        (("q", 1, 0), 21.2),
        (("k", 0, 1), 23.0),
        (("v", 4), 25.3),
        (("v", 5), 25.4),
        (("k", 1, 1), 25.6),
        (("v", 6), 25.8),
        (("v", 7), 25.9),
        (("k", 0, 2), 28.0),
        (("v", 8), 30.3),
        (("v", 9), 30.4),
        (("k", 1, 2), 30.6),
        (("v", 10), 30.8),
        (("v", 11), 30.9),
        (("k", 0, 3), 33.0),
        (("v", 12), 35.3),
        (("v", 13), 35.4),
        (("v", 14), 35.5),
        (("v", 15), 35.6),
        (("k", 1, 3), 36.0),
        (("q", 0, 1), 38.2),
        (("q", 1, 1), 40.2),
        (("q", 0, 2), 40.7),
        (("q", 1, 2), 42.7),
        (("q", 0, 3), 43.2),
        (("q", 1, 3), 45.2),
    ]
    for key, us in UNIT_STAMPS:
        if key[0] == "v":
            stamped(us, lambda key=key: v_unit(key[1]))
        else:
            stamped(us, lambda key=key: qk_unit(*key))

    # ---- attention: 8 software-pipelined k-sweeps -------------------------
    sweeps = [(qc, pair) for qc in range(NQC) for pair in range(2)]
    rz2 = {}
    u64 = {}
    prev = None  # (qc, pair, U, pt, kg) awaiting its PV + (for kg=7) evac

    def do_pv(qc, pair, U, pt, kg):
        for i in (0, 1):
            h = 2 * pair + i
            for kk in range(2):
                kt_ = kg * 2 + kk
                nc.tensor.matmul(
                    U[i][0:65, :],
                    (Vt[kt_][:, h, :]),
                    (pt[i][:, kk, :]),
                    start=(kg == 0 and kk == 0),
                    stop=(kg == NKT // 2 - 1 and kk == 1),
                )

    def do_evac(qc, pair, U):
        # z rows (partition 64 of U) -> partitions 0-1 of z2 via DMA; u64
        # copies free the U banks BEFORE the slow reciprocal occupies DVE.
        z2 = sb.tile([2, QC], F32, tag="z2", bufs=3, name=f"z2_{qc}{pair}")
        for i in (0, 1):
            zs = sb.tile([65, QC], F32, tag="zs", bufs=2, name=f"zs{qc}{pair}{i}")
            nc.vector.tensor_copy(zs[64:65, :], U[i][64:65, :])
            nc.sync.dma_start(z2[i : i + 1, :], zs[64:65, :])
        for i in (0, 1):
            t = sb.tile([64, QC], BF16, tag="u64", bufs=4, name=f"u64_{qc}{pair}{i}")
            nc.vector.tensor_copy(t[:], U[i][0:64, :])
            u64[pair, i] = t
        t = sb.tile([2, QC], BF16, tag="rz2", bufs=3, name=f"rz2_{qc}{pair}")
        nc.vector.reciprocal(t[:], z2[:])
        rz2[qc, pair] = t

    def do_norm(qc, tail=False):
        for pair in range(2):
            UN[qc, pair] = sb.tile([P, QC], BF16, tag="un", bufs=6, name=f"UN{qc}{pair}")
            for i in (0, 1):
                if i == 0:
                    r0 = rz2[qc, pair][0:1, :]
                else:
                    r0t = sb.tile(
                        [1, QC], BF16, tag="r0", bufs=3, name=f"r0_{qc}_{pair}{i}"
                    )
                    nc.sync.dma_start(r0t[:], rz2[qc, pair][1:2, :])
                    r0 = r0t[:]
                rb = sb.tile([64, QC], BF16, tag="rb", bufs=4, name=f"rb{qc}{pair}{i}")
                nc.gpsimd.partition_broadcast(rb[:], r0, channels=64)
                if i == 0:
                    nc.vector.tensor_mul(UN[qc, pair][0:64, :], u64[pair, i][:], rb[:])
                else:
                    tmp = sb.tile(
                        [64, QC], BF16, tag="untmp", bufs=2, name=f"untmp{qc}{pair}"
                    )
                    nc.vector.tensor_mul(tmp[:], u64[pair, i][:], rb[:])
                    nc.sync.dma_start(UN[qc, pair][64:128, :], tmp[:])

    STPRI = 1_000_000
    PVPRI = 1_000_000

    for si, (qc, pair) in enumerate(sweeps):
        pr = pair
        U = {
            i: ps.tile([P, QC], F32, tag="u", bufs=2, name=f"U{qc}_{pair}_{i}")
            for i in (0, 1)
        }
        for kg in range(NKT // 2):
            norm_qc = None
            with tc.high_priority(offset=STPRI):
                st = {
                    i: ps.tile(
                        [P, 2, QC], F32, tag="st", bufs=2, name=f"st{qc}{pair}{kg}{i}"
                    )
                    for i in (0, 1)
                }
                # scores^T: two heads on PE row-groups 0/64, adjacent issue ->
                # concurrent execution (64-contraction each).
                for kk in range(2):
                    kt_ = kg * 2 + kk
                    sc, off = divmod(kt_, 4)
                    for i in (0, 1):
                        lo = i * 64
                        nc.tensor.matmul(
                            st[i][:, kk, :],
                            (KT[pr, sc][lo : lo + 64, off * P : (off + 1) * P]),
                            (QT[pr, qc][lo : lo + 64, :]),
                            start=True,
                            stop=True,
                            tile_position=(lo, 0),
                        )
                pt = {
                    i: sb.tile(
                        [P, 2, QC], BF16, tag="pt", bufs=9, name=f"pt{qc}{pair}{kg}{i}"
                    )
                    for i in (0, 1)
                }
                for i in (0, 1):
                    nc.scalar.activation(pt[i][:], st[i][:], AF.Exp, scale=SCALE)
            # previous tick's PV (pipelined one tick behind ST/exp)
            with tc.high_priority(offset=PVPRI):
                if prev is not None:
                    pqc, ppair, pU, ppt, pkg = prev
                    do_pv(pqc, ppair, pU, ppt, pkg)
                    if pkg == NKT // 2 - 1:
                        do_evac(pqc, ppair, pU)
                        if ppair == 1:
                            do_norm(pqc)
                            norm_qc = pqc
            prev = (qc, pair, U, pt, kg)
            if norm_qc is not None:
                for qi in range(4):
                    for ec in range(2):
                        outproj_unit(norm_qc * 4 + qi, ec)

    # drain: final PV, evac, norm, out-projection of the last q-chunk
    pqc, ppair, pU, ppt, pkg = prev
    with tc.high_priority(offset=PVPRI):
        do_pv(pqc, ppair, pU, ppt, pkg)
        do_evac(pqc, ppair, pU)
        do_norm(pqc, tail=True)
    # dummy matmuls keep the PE activity monitor warm through the norm
    # chain so the final out-projections run at 2.4 GHz, not 1.2.
    wps = ps.tile([P, QC], F32, tag="u", bufs=2, name="tailwarm")
    for i in range(16):
        nc.tensor.matmul(
            wps[:], KT[0, 0][:, 0:P], KT[0, 0][:], start=(i == 0), stop=(i == 15)
        )
    for qi in range(4):
        for ec in range(2):
            outproj_unit(pqc * 4 + qi, ec, use_act=True)


def build_program():
    nc = bacc.Bacc(
        "TRN2", target_bir_lowering=False, debug=False, num_devices=NCORES
    )
    io = {
        "xq": nc.dram_tensor("xq", [NSB, P, CD, QC], BF16, kind="ExternalInput").ap(),
        "xk": nc.dram_tensor("xk", [NSB, P, CD, QC], BF16, kind="ExternalInput").ap(),
        "xv": nc.dram_tensor("xv", [NSB, P, CD, QC], BF16, kind="ExternalInput").ap(),
        "wq": nc.dram_tensor("wq", [P, CD, DG], BF16, kind="ExternalInput").ap(),
        "wk": nc.dram_tensor("wk", [P, CD, DG], BF16, kind="ExternalInput").ap(),
        "wv": nc.dram_tensor("wv", [P, CD, DG], BF16, kind="ExternalInput").ap(),
        "wo": nc.dram_tensor("wo", [DG, D], BF16, kind="ExternalInput").ap(),
        "bq": nc.dram_tensor("bq", [P, 2], F32, kind="ExternalInput").ap(),
        "bk": nc.dram_tensor("bk", [P, 2], F32, kind="ExternalInput").ap(),
        "ones4": nc.dram_tensor("ones4", [P, 4], BF16, kind="ExternalInput").ap(),
        "y": nc.dram_tensor("y", [S, D], BF16, kind="ExternalOutput").ap(),
    }
    with tile.TileContext(nc) as tc:
        with ExitStack() as ctx:
            _body(ctx, tc, io)
    nc.compile()
    return nc


_CACHE = {}


def _get_program():
    if "nc" not in _CACHE:
        _CACHE["nc"] = build_program()
    return _CACHE["nc"]


def make_in_maps(inputs):
    q = np.asarray(inputs["query"], np.float32)
    k = np.asarray(inputs["key"], np.float32)
    v = np.asarray(inputs["value"], np.float32)
    W_q = np.asarray(inputs["W_q"], np.float32)
    W_k = np.asarray(inputs["W_k"], np.float32)
    W_v = np.asarray(inputs["W_v"], np.float32)
    W_o = np.asarray(inputs["W_o"], np.float32)
    b_q = np.asarray(inputs["b_q"], np.float32)
    b_k = np.asarray(inputs["b_k"], np.float32)

    bf = ml_dtypes.bfloat16

    def xblocks(x, b):
        # [S, D] -> [sb, p, c, s] s-major 1MB blocks
        return np.ascontiguousarray(
            x[b].T.reshape(CD, P, NSB, QC).transpose(2, 1, 0, 3)
        ).astype(bf)

    def wblocks(W, sl):
        # W[sl, :].T -> [p, c, d]
        return np.ascontiguousarray(
            W[sl, :].T.reshape(CD, P, DG).transpose(1, 0, 2)
        ).astype(bf)

    def bblocks(bvec, sl):
        # [DG] -> [p, t]: bias for dim t*128+p
        return np.ascontiguousarray(bvec[sl].reshape(2, P).T)

    xb = [[xblocks(x, b) for b in range(B)] for x in (q, k, v)]
    in_maps = []
    for core in range(NCORES):
        b, g = divmod(core, NG)
        sl = slice(g * DG, (g + 1) * DG)
        in_maps.append(
            {
                "xq": xb[0][b],
                "xk": xb[1][b],
                "xv": xb[2][b],
                "wq": wblocks(W_q, sl),
                "wk": wblocks(W_k, sl),
                "wv": wblocks(W_v, sl),
                "wo": np.ascontiguousarray(W_o[:, sl].T).astype(bf),
                "bq": bblocks(b_q, sl),
                "bk": bblocks(b_k, sl),
                "ones4": np.ones((P, 4), bf),
            }
        )
    return in_maps


def kernel(**inputs):
    from concourse.bass_utils import run_bass_kernel_spmd

    nc = _get_program()
    in_maps = make_in_maps(inputs)
    trace = bool(int(os.environ.get("MHA_TRACE", "0")))
    res = run_bass_kernel_spmd(nc, in_maps, list(range(NCORES)), trace=trace)
    _CACHE["last_results"] = res

    W_o = np.asarray(inputs["W_o"], np.float32)
    b_v = np.asarray(inputs["b_v"], np.float32)
    b_o = np.asarray(inputs["b_o"], np.float32)
    out = np.zeros((B, S, D), np.float32)
    for core in range(NCORES):
        b = core // NG
        out[b] += res.results[core]["y"].astype(np.float32)
    out += (W_o @ b_v + b_o)[None, None, :]
    return out


# revision 43
# speedup vs baseline: 1.1835x; 1.1835x over previous
"""Multi-head attention (B=2, S=2048, D=1024, H=16) on 8 Trainium2 cores.

Sharding: core = 4*b + g  (b = batch 0..1, g = head-group 0..3, 4 heads each).

Single fused pipeline per core:
  - inputs host-relaid to s-major 1MB blocks [sb, p, c, s]; all input DMA
    drains through one HWDGE FIFO at ~400GB/s with ~7us startup, so every
    DMA-gated compute unit is stamped with tile_wait_until at its realistic
    arrival time — this keeps the Tile scheduler's static per-engine order
    feasible at runtime (its own DMA model has no shared-bandwidth cap);
  - attention runs per (q-chunk, head-pair) k-sweeps, software-pipelined at
    creation: tick t emits ST+exp(t) then PV(t-1), so the next sweep's first
    scores overlap the previous sweep's last PV drain (no ACT bubble);
  - ST = scores^T via 64-contraction matmuls, the two heads of a pair issued
    adjacently on PE row-groups (0,0)/(64,0) -> concurrent execution;
  - exp on ACT is the steady-state bottleneck (128 instrs x ~1.11us);
  - PV accumulates U_h [65, q] in PSUM (row 64 = softmax denominator via a
    ones column in V_aug); U evacuated to SBUF right after each sweep so the
    2 U banks recycle; reciprocals batched per q-chunk;
  - projection / out-projection units interleave as PE filler (HAM stays
    warm); V bias folded out algebraically (host adds W_o@b_v + b_o).

All matmuls in bfloat16 (1 cycle/row, FWL weight loads); fp32 PSUM accum.
"""

import os
from contextlib import ExitStack

import ml_dtypes
import numpy as np

import concourse.bass as bass
import concourse.tile as tile
from concourse import bacc, mybir

B, S, D = 2, 2048, 1024
H, DH = 16, 64
NCORES = 8
NG = 4                  # head-group shards
DG = D // NG            # 256 dims per head-group (4 heads)
P = 128
QC = 512                # q-chunk width
NQC = S // QC           # 4
NKT = S // P            # 16 k-tiles of 128
NSB = S // QC           # 4 s-blocks per input
CD = D // P             # 8 contraction chunks
F32 = mybir.dt.float32
BF16 = mybir.dt.bfloat16
AF = mybir.ActivationFunctionType
SCALE = 1.0 / float(np.sqrt(D))

# DMA arrival estimates (us): single FIFO queue, ~0.4 MB/us, ~7us startup.
_T0, _RATE = 7.0, 0.4


def _body(ctx: ExitStack, tc: "tile.TileContext", io: dict):
    nc = tc.nc
    ctx.enter_context(nc.allow_low_precision(reason="bf16 matmul pipeline"))
    sb = ctx.enter_context(tc.tile_pool(name="sb", bufs=1))
    ps = ctx.enter_context(tc.tile_pool(name="ps", bufs=1, space="PSUM"))

    # ---- input DMAs in FIFO priority order; track cumulative-MB ETAs ------
    eta = {}
    cum = [0.0]

    def ldma(nm, shape, dt, src, mb, nsplit=1):
        t = sb.tile(shape, dt, tag=nm, bufs=1, name=nm)
        if nsplit == 1:
            nc.sync.dma_start(t[:], src)
        else:
            step = shape[1] // nsplit
            for i in range(nsplit):
                sl = slice(i * step, (i + 1) * step)
                nc.sync.dma_start(t[:, sl], src[:, sl])
        cum[0] += mb
        eta[nm] = _T0 + cum[0] / _RATE
        return t

    ones4 = ldma("ones4", [P, 4], BF16, io["ones4"], 0.01)
    wk = ldma("wk", [P, CD, DG], BF16, io["wk"], 0.5)
    xk_t = [ldma("xk0", [P, CD, QC], BF16, io["xk"][0], 1.0, nsplit=2)]
    wq = ldma("wq", [P, CD, DG], BF16, io["wq"], 0.5)
    xq_t = [ldma("xq0", [P, CD, QC], BF16, io["xq"][0], 1.0, nsplit=2)]
    bk = ldma("bk", [P, 2], F32, io["bk"], 0.01)
    bq = ldma("bq", [P, 2], F32, io["bq"], 0.01)
    for b_ in range(1, NSB):
        xk_t.append(ldma(f"xk{b_}", [P, CD, QC], BF16, io["xk"][b_], 1.0))
    wv = ldma("wv", [P, CD, DG], BF16, io["wv"], 0.5)
    xv_t = [ldma(f"xv{b_}", [P, CD, QC], BF16, io["xv"][b_], 1.0) for b_ in range(NSB)]
    for b_ in range(1, NSB):
        xq_t.append(ldma(f"xq{b_}", [P, CD, QC], BF16, io["xq"][b_], 1.0))
    woT = [
        ldma(f"wo{pr}", [P, D], BF16, io["wo"][pr * P : (pr + 1) * P, :], 0.25)
        for pr in range(2)
    ]

    # ACT table preload: tiny exp on the first-arriving tile triggers the
    # one-time ~2.7us ACT_TABLE_LOAD while DMAs are still streaming.
    scr = sb.tile([P, 4], BF16, tag="scr", bufs=1, name="scr")
    nc.scalar.activation(scr[:], ones4[:], AF.Exp, scale=SCALE)

    QT, KT, Vt, UN, YSB = {}, {}, {}, {}, {}

    # ---- PE filler units, stamped with realistic DMA-arrival times --------
    def qk_unit(which, d, sc):
        w, xs, bias, outmap = (
            (wq, xq_t, bq, QT) if which == "q" else (wk, xk_t, bk, KT)
        )
        pg = ps.tile([P, QC], F32, tag="fil", bufs=2, name=f"pg_{which}{d}{sc}")
        for c in range(CD):
            nc.tensor.matmul(
                pg[:],
                (w[:, c, d * P : (d + 1) * P]),
                (xs[sc][:, c, :]),
                start=(c == 0),
                stop=(c == CD - 1),
            )
        t = sb.tile([P, QC], BF16, tag=f"{which}t", bufs=8, name=f"{which}T{d}{sc}")
        nc.vector.tensor_scalar_add(t[:], pg[:], bias[:, d : d + 1])
        outmap[d, sc] = t

    def v_unit(kt):
        blk, off = divmod(kt, 4)
        pg = ps.tile([P, DG], F32, tag="fil", bufs=2, name=f"pg_v{kt}")
        for c in range(CD):
            nc.tensor.matmul(
                pg[:],
                (xv_t[blk][:, c, off * P : (off + 1) * P]),
                (wv[:, c, :]),
                start=(c == 0),
                stop=(c == CD - 1),
            )
        vt = sb.tile([P, 4, DH + 1], BF16, tag="v", bufs=16, name=f"V{kt}")
        nc.vector.tensor_copy(
            vt[:, :, 0:DH], pg[:].rearrange("p (g d) -> p g d", g=4)
        )
        nc.vector.tensor_copy(vt[:, :, DH : DH + 1], ones4[:, :, None])
        Vt[kt] = vt

    def outproj_unit(qt, ec, use_act=False):
        qcp, qi = divmod(qt, 4)
        if ec == 0:
            YSB[qt] = sb.tile([P, D], BF16, tag="y", bufs=3, name=f"Y{qt}")
        ysb = YSB[qt]
        yp = ps.tile([P, QC], F32, tag="fil", bufs=2, name=f"yp{qt}_{ec}")
        for pr in range(2):
            nc.tensor.matmul(
                yp[:],
                (UN[qcp, pr][:, qi * P : (qi + 1) * P]),
                (woT[pr][:, ec * QC : (ec + 1) * QC]),
                start=(pr == 0),
                stop=(pr == 1),
            )
        if use_act:
            # tail: ACT is idle after the last exp; offload the PSUM->SBUF
            # evacuation there so DVE and ACT drain the tail in parallel.
            nc.scalar.activation(ysb[:, ec * QC : (ec + 1) * QC], yp[:], AF.Copy)
        else:
            nc.vector.tensor_copy(ysb[:, ec * QC : (ec + 1) * QC], yp[:])
        if ec == 1:
            nc.sync.dma_start(io["y"][qt * P : (qt + 1) * P, :], ysb[:])

    def stamped(us, fn):
        with tc.tile_wait_until(us / 1000.0):
            fn()

    # All units created upfront; stamps = max(DMA arrival, consumer-need
    # minus slack) so the scheduler neither front-loads them into the
    # exp-chain ramp nor bunches them ahead of late DMA arrivals.
    # create all projection units upfront; wait-stamps place them in the
    # scheduler's timeline at their realistic earliest-start times. All units
    # share the 2-slot "fil" PSUM tag whose rotation follows creation order.
    stamped(eta["xk0"], lambda: qk_unit("k", 0, 0))
    stamped(eta["xq0"], lambda: qk_unit("q", 0, 0))
    stamped(eta["xk0"] + 0.5, lambda: qk_unit("k", 1, 0))
    stamped(eta["xq0"] + 0.5, lambda: qk_unit("q", 1, 0))
    for sc in range(1, NSB):
        stamped(eta[f"xk{sc}"], lambda sc=sc: qk_unit("k", 0, sc))
        stamped(eta[f"xk{sc}"] + 0.5, lambda sc=sc: qk_unit("k", 1, sc))
    for kt in range(NKT):
        stamped(eta[f"xv{kt // 4}"] + 0.2, lambda kt=kt: v_unit(kt))
    for qc in range(1, NQC):
        for d in range(2):
            stamped(eta[f"xq{qc}"] + 0.2, lambda d=d, qc=qc: qk_unit("q", d, qc))

    # ---- attention: 8 software-pipelined k-sweeps -------------------------
    sweeps = [(qc, pair) for qc in range(NQC) for pair in range(2)]
    z4 = {}
    u64 = {}
    prev = None  # (qc, pair, U, pt, kg) awaiting its PV + (for kg=7) evac

    def do_pv(qc, pair, U, pt, kg):
        for i in (0, 1):
            h = 2 * pair + i
            for kk in range(2):
                kt_ = kg * 2 + kk
                nc.tensor.matmul(
                    U[i][0:65, :],
                    (Vt[kt_][:, h, :]),
                    (pt[i][:, kk, :]),
                    start=(kg == 0 and kk == 0),
                    stop=(kg == NKT // 2 - 1 and kk == 1),
                )

    def do_evac(qc, pair, U):
        for i in (0, 1):
            zs = sb.tile([65, QC], F32, tag="zs", bufs=2, name=f"zs{qc}{pair}{i}")
            nc.vector.tensor_copy(zs[64:65, :], U[i][64:65, :])
            j = 2 * pair + i
            nc.sync.dma_start(z4[qc][j : j + 1, :], zs[64:65, :])
        for i in (0, 1):
            t = sb.tile([64, QC], BF16, tag="u64", bufs=4, name=f"u64_{qc}{pair}{i}")
            nc.vector.tensor_copy(t[:], U[i][0:64, :])
            u64[pair, i] = t

    def do_norm(qc, tail=False):
        rz4 = sb.tile([4, QC], BF16, tag="rz4", bufs=2, name=f"rz4_{qc}")
        nc.vector.reciprocal(rz4[:], z4[qc][:])
        for pair in range(2):
            UN[qc, pair] = sb.tile([P, QC], BF16, tag="un", bufs=6, name=f"UN{qc}{pair}")
            for i in (0, 1):
                j = 2 * pair + i
                if j == 0:
                    r0 = rz4[0:1, :]
                else:
                    r0t = sb.tile([1, QC], BF16, tag="r0", bufs=3, name=f"r0_{qc}_{j}")
                    nc.sync.dma_start(r0t[:], rz4[j : j + 1, :])
                    r0 = r0t[:]
                rb = sb.tile([64, QC], BF16, tag="rb", bufs=4, name=f"rb{qc}{pair}{i}")
                nc.gpsimd.partition_broadcast(rb[:], r0, channels=64)
                if i == 0:
                    nc.vector.tensor_mul(UN[qc, pair][0:64, :], u64[pair, i][:], rb[:])
                else:
                    tmp = sb.tile(
                        [64, QC], BF16, tag="untmp", bufs=2, name=f"untmp{qc}{pair}"
                    )
                    nc.vector.tensor_mul(tmp[:], u64[pair, i][:], rb[:])
                    nc.sync.dma_start(UN[qc, pair][64:128, :], tmp[:])

    STPRI = 1_000_000
    PVPRI = 1_000_000

    for si, (qc, pair) in enumerate(sweeps):
        pr = pair
        if pair == 0:
            z4[qc] = sb.tile([4, QC], F32, tag="z4", bufs=2, name=f"z4_{qc}")
        U = {
            i: ps.tile([P, QC], F32, tag="u", bufs=2, name=f"U{qc}_{pair}_{i}")
            for i in (0, 1)
        }
        for kg in range(NKT // 2):
            norm_qc = None
            with tc.high_priority(offset=STPRI):
                st = {
                    i: ps.tile(
                        [P, 2, QC], F32, tag="st", bufs=2, name=f"st{qc}{pair}{kg}{i}"
                    )
                    for i in (0, 1)
                }
                # scores^T: two heads on PE row-groups 0/64, adjacent issue ->
                # concurrent execution (64-contraction each).
                for kk in range(2):
                    kt_ = kg * 2 + kk
                    sc, off = divmod(kt_, 4)
                    for i in (0, 1):
                        lo = i * 64
                        nc.tensor.matmul(
                            st[i][:, kk, :],
                            (KT[pr, sc][lo : lo + 64, off * P : (off + 1) * P]),
                            (QT[pr, qc][lo : lo + 64, :]),
                            start=True,
                            stop=True,
                            tile_position=(lo, 0),
                        )
                pt = {
                    i: sb.tile(
                        [P, 2, QC], BF16, tag="pt", bufs=9, name=f"pt{qc}{pair}{kg}{i}"
                    )
                    for i in (0, 1)
                }
                for i in (0, 1):
                    nc.scalar.activation(pt[i][:], st[i][:], AF.Exp, scale=SCALE)
            # previous tick's PV (pipelined one tick behind ST/exp)
            with tc.high_priority(offset=PVPRI):
                if prev is not None:
                    pqc, ppair, pU, ppt, pkg = prev
                    do_pv(pqc, ppair, pU, ppt, pkg)
                    if pkg == NKT // 2 - 1:
                        do_evac(pqc, ppair, pU)
                        if ppair == 1:
                            do_norm(pqc)
                            norm_qc = pqc
            prev = (qc, pair, U, pt, kg)
            if norm_qc is not None:
                for qi in range(4):
                    for ec in range(2):
                        outproj_unit(norm_qc * 4 + qi, ec)

    # drain: final PV, evac, norm, out-projection of the last q-chunk
    pqc, ppair, pU, ppt, pkg = prev
    with tc.high_priority(offset=PVPRI):
        do_pv(pqc, ppair, pU, ppt, pkg)
        do_evac(pqc, ppair, pU)
        do_norm(pqc, tail=True)
    # dummy matmuls keep the PE activity monitor warm through the norm
    # chain so the final out-projections run at 2.4 GHz, not 1.2.
    wps = ps.tile([P, QC], F32, tag="u", bufs=2, name="tailwarm")
    for i in range(16):
        nc.tensor.matmul(
            wps[:], KT[0, 0][:, 0:P], KT[0, 0][:], start=(i == 0), stop=(i == 15)
        )
    for qi in range(4):
        for ec in range(2):
            outproj_unit(pqc * 4 + qi, ec, use_act=True)


def build_program():
    nc = bacc.Bacc(
        "TRN2", target_bir_lowering=False, debug=False, num_devices=NCORES
    )
    io = {
        "xq": nc.dram_tensor("xq", [NSB, P, CD, QC], BF16, kind="ExternalInput").ap(),
        "xk": nc.dram_tensor("xk", [NSB, P, CD, QC], BF16, kind="ExternalInput").ap(),
        "xv": nc.dram_tensor("xv", [NSB, P, CD, QC], BF16, kind="ExternalInput").ap(),
        "wq": nc.dram_tensor("wq", [P, CD, DG], BF16, kind="ExternalInput").ap(),
        "wk": nc.dram_tensor("wk", [P, CD, DG], BF16, kind="ExternalInput").ap(),
        "wv": nc.dram_tensor("wv", [P, CD, DG], BF16, kind="ExternalInput").ap(),
        "wo": nc.dram_tensor("wo", [DG, D], BF16, kind="ExternalInput").ap(),
        "bq": nc.dram_tensor("bq", [P, 2], F32, kind="ExternalInput").ap(),
        "bk": nc.dram_tensor("bk", [P, 2], F32, kind="ExternalInput").ap(),
        "ones4": nc.dram_tensor("ones4", [P, 4], BF16, kind="ExternalInput").ap(),
        "y": nc.dram_tensor("y", [S, D], BF16, kind="ExternalOutput").ap(),
    }
    with tile.TileContext(nc) as tc:
        with ExitStack() as ctx:
            _body(ctx, tc, io)
    nc.compile()
    return nc


_CACHE = {}


def _get_program():
    if "nc" not in _CACHE:
        _CACHE["nc"] = build_program()
    return _CACHE["nc"]


def make_in_maps(inputs):
    q = np.asarray(inputs["query"], np.float32)
    k = np.asarray(inputs["key"], np.float32)
    v = np.asarray(inputs["value"], np.float32)
    W_q = np.asarray(inputs["W_q"], np.float32)
    W_k = np.asarray(inputs["W_k"], np.float32)
    W_v = np.asarray(inputs["W_v"], np.float32)
    W_o = np.asarray(inputs["W_o"], np.float32)
    b_q = np.asarray(inputs["b_q"], np.float32)
    b_k = np.asarray(inputs["b_k"], np.float32)

    bf = ml_dtypes.bfloat16

    def xblocks(x, b):
        # [S, D] -> [sb, p, c, s] s-major 1MB blocks
        return np.ascontiguousarray(
            x[b].T.reshape(CD, P, NSB, QC).transpose(2, 1, 0, 3)
        ).astype(bf)

    def wblocks(W, sl):
        # W[sl, :].T -> [p, c, d]
        return np.ascontiguousarray(
            W[sl, :].T.reshape(CD, P, DG).transpose(1, 0, 2)
        ).astype(bf)

    def bblocks(bvec, sl):
        # [DG] -> [p, t]: bias for dim t*128+p
        return np.ascontiguousarray(bvec[sl].reshape(2, P).T)

    xb = [[xblocks(x, b) for b in range(B)] for x in (q, k, v)]
    in_maps = []
    for core in range(NCORES):
        b, g = divmod(core, NG)
        sl = slice(g * DG, (g + 1) * DG)
        in_maps.append(
            {
                "xq": xb[0][b],
                "xk": xb[1][b],
                "xv": xb[2][b],
                "wq": wblocks(W_q, sl),
                "wk": wblocks(W_k, sl),
                "wv": wblocks(W_v, sl),
                "wo": np.ascontiguousarray(W_o[:, sl].T).astype(bf),
                "bq": bblocks(b_q, sl),
                "bk": bblocks(b_k, sl),
                "ones4": np.ones((P, 4), bf),
            }
        )
    return in_maps


def kernel(**inputs):
    from concourse.bass_utils import run_bass_kernel_spmd

    nc = _get_program()
    in_maps = make_in_maps(inputs)
    trace = bool(int(os.environ.get("MHA_TRACE", "0")))
    res = run_bass_kernel_spmd(nc, in_maps, list(range(NCORES)), trace=trace)
    _CACHE["last_results"] = res

    W_o = np.asarray(inputs["W_o"], np.float32)
    b_v = np.asarray(inputs["b_v"], np.float32)
    b_o = np.asarray(inputs["b_o"], np.float32)
    out = np.zeros((B, S, D), np.float32)
    for core in range(NCORES):
        b = core // NG
        out[b] += res.results[core]["y"].astype(np.float32)
    out += (W_o @ b_v + b_o)[None, None, :]
    return out


# revision 46
# speedup vs baseline: 1.1856x; 1.0017x over previous
"""Multi-head attention (B=2, S=2048, D=1024, H=16) on 8 Trainium2 cores.

Sharding: core = 4*b + g  (b = batch 0..1, g = head-group 0..3, 4 heads each).

Single fused pipeline per core:
  - inputs host-relaid to s-major 1MB blocks [sb, p, c, s]; all input DMA
    drains through one HWDGE FIFO at ~400GB/s with ~7us startup, so every
    DMA-gated compute unit is stamped with tile_wait_until at its realistic
    arrival time — this keeps the Tile scheduler's static per-engine order
    feasible at runtime (its own DMA model has no shared-bandwidth cap);
  - attention runs per (q-chunk, head-pair) k-sweeps, software-pipelined at
    creation: tick t emits ST+exp(t) then PV(t-1), so the next sweep's first
    scores overlap the previous sweep's last PV drain (no ACT bubble);
  - ST = scores^T via 64-contraction matmuls, the two heads of a pair issued
    adjacently on PE row-groups (0,0)/(64,0) -> concurrent execution;
  - exp on ACT is the steady-state bottleneck (128 instrs x ~1.11us);
  - PV accumulates U_h [65, q] in PSUM (row 64 = softmax denominator via a
    ones column in V_aug); U evacuated to SBUF right after each sweep so the
    2 U banks recycle; reciprocals batched per q-chunk;
  - projection / out-projection units interleave as PE filler (HAM stays
    warm); V bias folded out algebraically (host adds W_o@b_v + b_o).

All matmuls in bfloat16 (1 cycle/row, FWL weight loads); fp32 PSUM accum.
"""

import os
from contextlib import ExitStack

import ml_dtypes
import numpy as np

import concourse.bass as bass
import concourse.tile as tile
from concourse import bacc, mybir

B, S, D = 2, 2048, 1024
H, DH = 16, 64
NCORES = 8
NG = 4                  # head-group shards
DG = D // NG            # 256 dims per head-group (4 heads)
P = 128
QC = 512                # q-chunk width
NQC = S // QC           # 4
NKT = S // P            # 16 k-tiles of 128
NSB = S // QC           # 4 s-blocks per input
CD = D // P             # 8 contraction chunks
F32 = mybir.dt.float32
BF16 = mybir.dt.bfloat16
AF = mybir.ActivationFunctionType
SCALE = 1.0 / float(np.sqrt(D))

# DMA arrival estimates (us): single FIFO queue, ~0.4 MB/us, ~7us startup.
_T0, _RATE = 7.0, 0.4


def _body(ctx: ExitStack, tc: "tile.TileContext", io: dict):
    nc = tc.nc
    ctx.enter_context(nc.allow_low_precision(reason="bf16 matmul pipeline"))
    sb = ctx.enter_context(tc.tile_pool(name="sb", bufs=1))
    ps = ctx.enter_context(tc.tile_pool(name="ps", bufs=1, space="PSUM"))

    # ---- input DMAs in FIFO priority order; track cumulative-MB ETAs ------
    eta = {}
    cum = [0.0]

    def ldma(nm, shape, dt, src, mb, nsplit=1):
        t = sb.tile(shape, dt, tag=nm, bufs=1, name=nm)
        if nsplit == 1:
            nc.sync.dma_start(t[:], src)
        else:
            step = shape[1] // nsplit
            for i in range(nsplit):
                sl = slice(i * step, (i + 1) * step)
                nc.sync.dma_start(t[:, sl], src[:, sl])
        cum[0] += mb
        eta[nm] = _T0 + cum[0] / _RATE
        return t

    ones4 = ldma("ones4", [P, 4], BF16, io["ones4"], 0.01)
    wk = ldma("wk", [P, CD, DG], BF16, io["wk"], 0.5)
    xk_t = [ldma("xk0", [P, CD, QC], BF16, io["xk"][0], 1.0, nsplit=2)]
    wq = ldma("wq", [P, CD, DG], BF16, io["wq"], 0.5)
    xq_t = [ldma("xq0", [P, CD, QC], BF16, io["xq"][0], 1.0, nsplit=2)]
    bk = ldma("bk", [P, 2], F32, io["bk"], 0.01)
    bq = ldma("bq", [P, 2], F32, io["bq"], 0.01)
    for b_ in range(1, NSB):
        xk_t.append(ldma(f"xk{b_}", [P, CD, QC], BF16, io["xk"][b_], 1.0))
    wv = ldma("wv", [P, CD, DG], BF16, io["wv"], 0.5)
    xv_t = [ldma(f"xv{b_}", [P, CD, QC], BF16, io["xv"][b_], 1.0) for b_ in range(NSB)]
    for b_ in range(1, NSB):
        xq_t.append(ldma(f"xq{b_}", [P, CD, QC], BF16, io["xq"][b_], 1.0))
    woT = [
        ldma(f"wo{pr}", [P, D], BF16, io["wo"][pr * P : (pr + 1) * P, :], 0.25)
        for pr in range(2)
    ]

    # ACT table preload: tiny exp on the first-arriving tile triggers the
    # one-time ~2.7us ACT_TABLE_LOAD while DMAs are still streaming.
    scr = sb.tile([P, 4], BF16, tag="scr", bufs=1, name="scr")
    nc.scalar.activation(scr[:], ones4[:], AF.Exp, scale=SCALE)

    QT, KT, Vt, UN, YSB = {}, {}, {}, {}, {}

    # ---- PE filler units, stamped with realistic DMA-arrival times --------
    def qk_unit(which, d, sc):
        w, xs, bias, outmap = (
            (wq, xq_t, bq, QT) if which == "q" else (wk, xk_t, bk, KT)
        )
        pg = ps.tile([P, QC], F32, tag="fil", bufs=2, name=f"pg_{which}{d}{sc}")
        for c in range(CD):
            nc.tensor.matmul(
                pg[:],
                (w[:, c, d * P : (d + 1) * P]),
                (xs[sc][:, c, :]),
                start=(c == 0),
                stop=(c == CD - 1),
            )
        t = sb.tile([P, QC], BF16, tag=f"{which}t", bufs=8, name=f"{which}T{d}{sc}")
        nc.vector.tensor_scalar_add(t[:], pg[:], bias[:, d : d + 1])
        outmap[d, sc] = t

    def v_unit(kt):
        blk, off = divmod(kt, 4)
        pg = ps.tile([P, DG], F32, tag="fil", bufs=2, name=f"pg_v{kt}")
        for c in range(CD):
            nc.tensor.matmul(
                pg[:],
                (xv_t[blk][:, c, off * P : (off + 1) * P]),
                (wv[:, c, :]),
                start=(c == 0),
                stop=(c == CD - 1),
            )
        vt = sb.tile([P, 4, DH + 1], BF16, tag="v", bufs=16, name=f"V{kt}")
        nc.vector.tensor_copy(
            vt[:, :, 0:DH], pg[:].rearrange("p (g d) -> p g d", g=4)
        )
        nc.vector.tensor_copy(vt[:, :, DH : DH + 1], ones4[:, :, None])
        Vt[kt] = vt

    def outproj_unit(qt, ec, use_act=False):
        qcp, qi = divmod(qt, 4)
        if ec == 0:
            YSB[qt] = sb.tile([P, D], BF16, tag="y", bufs=4, name=f"Y{qt}")
        ysb = YSB[qt]
        yp = ps.tile([P, QC], F32, tag="fil", bufs=2, name=f"yp{qt}_{ec}")
        for pr in range(2):
            nc.tensor.matmul(
                yp[:],
                (UN[qcp, pr][:, qi * P : (qi + 1) * P]),
                (woT[pr][:, ec * QC : (ec + 1) * QC]),
                start=(pr == 0),
                stop=(pr == 1),
            )
        if use_act:
            # tail: ACT is idle after the last exp; offload the PSUM->SBUF
            # evacuation there so DVE and ACT drain the tail in parallel.
            nc.scalar.activation(ysb[:, ec * QC : (ec + 1) * QC], yp[:], AF.Copy)
        else:
            nc.vector.tensor_copy(ysb[:, ec * QC : (ec + 1) * QC], yp[:])
        if ec == 1:
            nc.sync.dma_start(io["y"][qt * P : (qt + 1) * P, :], ysb[:])

    def stamped(us, fn):
        with tc.tile_wait_until(us / 1000.0):
            fn()

    # All units created upfront; stamps = max(DMA arrival, consumer-need
    # minus slack) so the scheduler neither front-loads them into the
    # exp-chain ramp nor bunches them ahead of late DMA arrivals.
    # create all projection units upfront; wait-stamps place them in the
    # scheduler's timeline at their realistic earliest-start times. All units
    # share the 2-slot "fil" PSUM tag whose rotation follows creation order.
    stamped(eta["xk0"], lambda: qk_unit("k", 0, 0))
    stamped(eta["xq0"], lambda: qk_unit("q", 0, 0))
    stamped(eta["xk0"] + 0.5, lambda: qk_unit("k", 1, 0))
    stamped(eta["xq0"] + 0.5, lambda: qk_unit("q", 1, 0))
    for sc in range(1, NSB):
        stamped(eta[f"xk{sc}"], lambda sc=sc: qk_unit("k", 0, sc))
        stamped(eta[f"xk{sc}"] + 0.5, lambda sc=sc: qk_unit("k", 1, sc))
    for kt in range(NKT):
        stamped(eta[f"xv{kt // 4}"] + 0.2, lambda kt=kt: v_unit(kt))
    for qc in range(1, NQC):
        for d in range(2):
            stamped(eta[f"xq{qc}"] + 0.2, lambda d=d, qc=qc: qk_unit("q", d, qc))

    # ---- attention: 8 software-pipelined k-sweeps -------------------------
    sweeps = [(qc, pair) for qc in range(NQC) for pair in range(2)]
    z4 = {}
    u64 = {}
    prev = None  # (qc, pair, U, pt, kg) awaiting its PV + (for kg=7) evac

    def do_pv(qc, pair, U, pt, kg):
        for i in (0, 1):
            h = 2 * pair + i
            for kk in range(2):
                kt_ = kg * 2 + kk
                nc.tensor.matmul(
                    U[i][0:65, :],
                    (Vt[kt_][:, h, :]),
                    (pt[i][:, kk, :]),
                    start=(kg == 0 and kk == 0),
                    stop=(kg == NKT // 2 - 1 and kk == 1),
                )

    def do_evac(qc, pair, U):
        for i in (0, 1):
            zs = sb.tile([65, QC], F32, tag="zs", bufs=2, name=f"zs{qc}{pair}{i}")
            nc.vector.tensor_copy(zs[64:65, :], U[i][64:65, :])
            j = 2 * pair + i
            nc.sync.dma_start(z4[qc][j : j + 1, :], zs[64:65, :])
        for i in (0, 1):
            t = sb.tile([64, QC], BF16, tag="u64", bufs=4, name=f"u64_{qc}{pair}{i}")
            nc.vector.tensor_copy(t[:], U[i][0:64, :])
            u64[pair, i] = t

    def do_norm(qc, tail=False):
        rz4 = sb.tile([4, QC], BF16, tag="rz4", bufs=2, name=f"rz4_{qc}")
        nc.vector.reciprocal(rz4[:], z4[qc][:])
        for pair in range(2):
            UN[qc, pair] = sb.tile([P, QC], BF16, tag="un", bufs=8, name=f"UN{qc}{pair}")
            for i in (0, 1):
                j = 2 * pair + i
                if j == 0:
                    r0 = rz4[0:1, :]
                else:
                    r0t = sb.tile([1, QC], BF16, tag="r0", bufs=3, name=f"r0_{qc}_{j}")
                    nc.sync.dma_start(r0t[:], rz4[j : j + 1, :])
                    r0 = r0t[:]
                rb = sb.tile([64, QC], BF16, tag="rb", bufs=4, name=f"rb{qc}{pair}{i}")
                nc.gpsimd.partition_broadcast(rb[:], r0, channels=64)
                if i == 0:
                    nc.vector.tensor_mul(UN[qc, pair][0:64, :], u64[pair, i][:], rb[:])
                else:
                    tmp = sb.tile(
                        [64, QC], BF16, tag="untmp", bufs=2, name=f"untmp{qc}{pair}"
                    )
                    nc.vector.tensor_mul(tmp[:], u64[pair, i][:], rb[:])
                    nc.sync.dma_start(UN[qc, pair][64:128, :], tmp[:])

    STPRI = 1_000_000
    PVPRI = 1_000_000

    for si, (qc, pair) in enumerate(sweeps):
        pr = pair
        if pair == 0:
            z4[qc] = sb.tile([4, QC], F32, tag="z4", bufs=2, name=f"z4_{qc}")
        U = {
            i: ps.tile([P, QC], F32, tag="u", bufs=2, name=f"U{qc}_{pair}_{i}")
            for i in (0, 1)
        }
        for kg in range(NKT // 2):
            norm_qc = None
            with tc.high_priority(offset=STPRI):
                st = {
                    i: ps.tile(
                        [P, 2, QC], F32, tag="st", bufs=2, name=f"st{qc}{pair}{kg}{i}"
                    )
                    for i in (0, 1)
                }
                # scores^T: two heads on PE row-groups 0/64, adjacent issue ->
                # concurrent execution (64-contraction each).
                for kk in range(2):
                    kt_ = kg * 2 + kk
                    sc, off = divmod(kt_, 4)
                    for i in (0, 1):
                        lo = i * 64
                        nc.tensor.matmul(
                            st[i][:, kk, :],
                            (KT[pr, sc][lo : lo + 64, off * P : (off + 1) * P]),
                            (QT[pr, qc][lo : lo + 64, :]),
                            start=True,
                            stop=True,
                            tile_position=(lo, 0),
                        )
                pt = {
                    i: sb.tile(
                        [P, 2, QC], BF16, tag="pt", bufs=9, name=f"pt{qc}{pair}{kg}{i}"
                    )
                    for i in (0, 1)
                }
                for i in (0, 1):
                    nc.scalar.activation(pt[i][:], st[i][:], AF.Exp, scale=SCALE)
            # previous tick's PV (pipelined one tick behind ST/exp)
            with tc.high_priority(offset=PVPRI):
                if prev is not None:
                    pqc, ppair, pU, ppt, pkg = prev
                    do_pv(pqc, ppair, pU, ppt, pkg)
                    if pkg == NKT // 2 - 1:
                        do_evac(pqc, ppair, pU)
                        if ppair == 1:
                            do_norm(pqc)
                            norm_qc = pqc
            prev = (qc, pair, U, pt, kg)
            if norm_qc is not None:
                for qi in range(4):
                    for ec in range(2):
                        outproj_unit(norm_qc * 4 + qi, ec)

    # drain: final PV, evac, norm, out-projection of the last q-chunk
    pqc, ppair, pU, ppt, pkg = prev
    with tc.high_priority(offset=PVPRI):
        do_pv(pqc, ppair, pU, ppt, pkg)
        do_evac(pqc, ppair, pU)
        do_norm(pqc, tail=True)
    for qi in range(4):
        for ec in range(2):
            outproj_unit(pqc * 4 + qi, ec, use_act=True)


def build_program():
    nc = bacc.Bacc(
        "TRN2", target_bir_lowering=False, debug=False, num_devices=NCORES
    )
    io = {
        "xq": nc.dram_tensor("xq", [NSB, P, CD, QC], BF16, kind="ExternalInput").ap(),
        "xk": nc.dram_tensor("xk", [NSB, P, CD, QC], BF16, kind="ExternalInput").ap(),
        "xv": nc.dram_tensor("xv", [NSB, P, CD, QC], BF16, kind="ExternalInput").ap(),
        "wq": nc.dram_tensor("wq", [P, CD, DG], BF16, kind="ExternalInput").ap(),
        "wk": nc.dram_tensor("wk", [P, CD, DG], BF16, kind="ExternalInput").ap(),
        "wv": nc.dram_tensor("wv", [P, CD, DG], BF16, kind="ExternalInput").ap(),
        "wo": nc.dram_tensor("wo", [DG, D], BF16, kind="ExternalInput").ap(),
        "bq": nc.dram_tensor("bq", [P, 2], F32, kind="ExternalInput").ap(),
        "bk": nc.dram_tensor("bk", [P, 2], F32, kind="ExternalInput").ap(),
        "ones4": nc.dram_tensor("ones4", [P, 4], BF16, kind="ExternalInput").ap(),
        "y": nc.dram_tensor("y", [S, D], BF16, kind="ExternalOutput").ap(),
    }
    with tile.TileContext(nc) as tc:
        with ExitStack() as ctx:
            _body(ctx, tc, io)
    nc.compile()
    return nc


_CACHE = {}


def _get_program():
    if "nc" not in _CACHE:
        _CACHE["nc"] = build_program()
    return _CACHE["nc"]


def make_in_maps(inputs):
    q = np.asarray(inputs["query"], np.float32)
    k = np.asarray(inputs["key"], np.float32)
    v = np.asarray(inputs["value"], np.float32)
    W_q = np.asarray(inputs["W_q"], np.float32)
    W_k = np.asarray(inputs["W_k"], np.float32)
    W_v = np.asarray(inputs["W_v"], np.float32)
    W_o = np.asarray(inputs["W_o"], np.float32)
    b_q = np.asarray(inputs["b_q"], np.float32)
    b_k = np.asarray(inputs["b_k"], np.float32)

    bf = ml_dtypes.bfloat16

    def xblocks(x, b):
        # [S, D] -> [sb, p, c, s] s-major 1MB blocks
        return np.ascontiguousarray(
            x[b].T.reshape(CD, P, NSB, QC).transpose(2, 1, 0, 3)
        ).astype(bf)

    def wblocks(W, sl):
        # W[sl, :].T -> [p, c, d]
        return np.ascontiguousarray(
            W[sl, :].T.reshape(CD, P, DG).transpose(1, 0, 2)
        ).astype(bf)

    def bblocks(bvec, sl):
        # [DG] -> [p, t]: bias for dim t*128+p
        return np.ascontiguousarray(bvec[sl].reshape(2, P).T)

    xb = [[xblocks(x, b) for b in range(B)] for x in (q, k, v)]
    in_maps = []
    for core in range(NCORES):
        b, g = divmod(core, NG)
        sl = slice(g * DG, (g + 1) * DG)
        in_maps.append(
            {
                "xq": xb[0][b],
                "xk": xb[1][b],
                "xv": xb[2][b],
                "wq": wblocks(W_q, sl),
                "wk": wblocks(W_k, sl),
                "wv": wblocks(W_v, sl),
                "wo": np.ascontiguousarray(W_o[:, sl].T).astype(bf),
                "bq": bblocks(b_q, sl),
                "bk": bblocks(b_k, sl),
                "ones4": np.ones((P, 4), bf),
            }
        )
    return in_maps


def kernel(**inputs):
    from concourse.bass_utils import run_bass_kernel_spmd

    nc = _get_program()
    in_maps = make_in_maps(inputs)
    trace = bool(int(os.environ.get("MHA_TRACE", "0")))
    res = run_bass_kernel_spmd(nc, in_maps, list(range(NCORES)), trace=trace)
    _CACHE["last_results"] = res

    W_o = np.asarray(inputs["W_o"], np.float32)
    b_v = np.asarray(inputs["b_v"], np.float32)
    b_o = np.asarray(inputs["b_o"], np.float32)
    out = np.zeros((B, S, D), np.float32)
    for core in range(NCORES):
        b = core // NG
        out[b] += res.results[core]["y"].astype(np.float32)
    out += (W_o @ b_v + b_o)[None, None, :]
    return out
